# revision 38
# baseline (speedup 1.0000x reference)
"""3-layer GAT (GATConv x3 + FC) on 8 Trainium2 NeuronCores.

Strategy: dst-sorted edge partitioning (each core owns a contiguous node range
and all edges into it), per-layer node-parallel feature matmul + AllGather of a
gatherable node table [h | e_src], then an edge phase per core: batched
dma_gather of src rows (int16 indices; the table is split into lo/hi halves so
indices fit in int16, and edges are grouped per (chunk, window, half)),
exp(leaky(e_src+e_dst)) edge weights, and segment reduction via one-hot
selection-matrix matmuls accumulated in PSUM per 128-node chunk.  Softmax is
unnormalized (exp without max subtraction) with per-node post-normalization by
the gathered weight sum; self-loops guarantee the sum stays well away from 0.
"""
import os, sys
sys.path.insert(0, '/opt/trn_rl_repo')
import math
import numpy as np
ATH_STAGE = int(os.environ.get("ATH_STAGE", "6"))
ATH_EDGE = int(os.environ.get("ATH_EDGE", "4"))
import ml_dtypes

import concourse.bass as bass
import concourse.bacc as bacc
import concourse.mybir as mybir
import concourse.tile as tile
from concourse import bass_utils
from concourse.bass import _add_dep_helper

# ---- model constants (must match reference.py) ----
NEG_SLOPE = 0.2
H1, H2, H3 = 4, 4, 1
CH = 64
N_NODES = 50000
N_EDGES = 800000
IN_DIM = 128
N_CLASSES = 10

W = 8                    # cores
OWN = 6272               # nodes per core (49 chunks of 128)
NPAD = W * OWN           # 50176
LOH = 3200               # within-core lo-half rows (25 chunks)
NLOROW = W * LOH         # 25600 rows in the lo table (< int16 max)
NHIROW = W * (OWN - LOH) # 24576 rows in the hi table
NCHUNK = OWN // 128      # 49
CPB = 3                  # chunks per gather block
NWIN = 2                 # 64-node windows per chunk
WINW = 64
PAD_SEG = 99.0

# table row layout (bf16 slots)
ROW12 = 384              # layers 1/2: [h(256) | e_src f32 (8 slots) | pad] = 768B stride
PAY12 = 264
ROW3 = 128               # layer 3: [h(64) | e_src f32 (2 slots) | pad] = 256B stride
PAY3 = 66
EROW = 128               # e_dst local table row: 256B stride, leading 8 (or 2) slots = f32

dt = mybir.dt
AF = mybir.ActivationFunctionType
OP = mybir.AluOpType
bf16 = ml_dtypes.bfloat16

# relax dma_gather's elem_size%256 assert (q7 ucode only requires the row
# stride to be a multiple of 256B; elem_size is the per-descriptor length)
import inspect as _inspect, textwrap as _textwrap
_src = _textwrap.dedent(_inspect.getsource(bass.BassGpSimd.dma_gather))
_src = _src.replace("elem_size_bytes > 0 and elem_size_bytes % 256 == 0",
                    "elem_size_bytes > 0")
_ns = dict(bass.__dict__)
exec(compile(_src, "<dma_gather_patched>", "exec"), _ns)
bass.BassGpSimd.dma_gather = _ns["dma_gather"]


def _wrap_idxs(idx_i16):
    """dma_gather index layout: idx i at [i%16, i//16], replicated to 128 parts."""
    n = idx_i16.shape[0]
    cols = n // 16
    arr = idx_i16.reshape(cols, 16).T.copy()
    return np.tile(arr, (8, 1))


def _host_prep(edge_index):
    """Edge sorting/tiling; returns per-core input arrays + static tile metadata.

    Edges (with self-loops) are grouped per (core, chunk, window, srchalf) and
    padded to 128-edge tiles; tile counts are the max over cores so the SPMD
    program is identical everywhere.  Per block (CPB chunks) the tile order is
    all lo-half tiles (groups in order) then all hi-half tiles.
    """
    src = np.concatenate([edge_index[0].astype(np.int64),
                          np.arange(N_NODES, dtype=np.int64)])
    dst = np.concatenate([edge_index[1].astype(np.int64),
                          np.arange(N_NODES, dtype=np.int64)])

    core = dst // OWN
    chunk = (dst % OWN) // 128
    win = ((dst % OWN) % 128) // WINW
    # lo/hi = within-core row halves (so each half AllGathers separately and
    # each table stays under the int16 index limit)
    half = ((src % OWN) >= LOH).astype(np.int64)
    srow = np.where(half == 0,
                    (src // OWN) * LOH + (src % OWN),
                    (src // OWN) * (OWN - LOH) + (src % OWN) - LOH)

    key = (((core * NCHUNK + chunk) * NWIN + win) * 2 + half)
    korder = np.argsort(key, kind="stable")
    src_s, dst_s, srow_s = src[korder], dst[korder], srow[korder]
    ngroups = W * NCHUNK * NWIN * 2
    counts = np.bincount(key[korder], minlength=ngroups).reshape(
        W, NCHUNK, NWIN, 2)
    starts = np.zeros(ngroups + 1, dtype=np.int64)
    np.cumsum(counts.reshape(-1), out=starts[1:])

    tiles_per = np.ceil(counts / 128.0).astype(np.int64).max(axis=0)  # [NCHUNK][NWIN][2]

    blocks = []
    c0 = 0
    while c0 < NCHUNK:
        nb = min(CPB, NCHUNK - c0)
        lo, hi = [], []
        for cl in range(nb):
            for w_ in range(NWIN):
                ntl = int(tiles_per[c0 + cl][w_][0])
                nth = int(tiles_per[c0 + cl][w_][1])
                if ntl:
                    lo.append((cl, w_, ntl))
                if nth:
                    hi.append((cl, w_, nth))
        Tlo = sum(t[2] for t in lo)
        Thi = sum(t[2] for t in hi)
        blocks.append(dict(c0=c0, nb=nb, lo=lo, hi=hi,
                           Tlo=Tlo, Thi=Thi, Tall=Tlo + Thi))
        c0 += nb

    per_core = []
    for r in range(W):
        ilo_cols, ihi_cols, ed_cols, seg_cols = [], [], [], []
        for b in blocks:
            for cls, tl, icols in ((0, b["lo"], ilo_cols), (1, b["hi"], ihi_cols)):
                for (cl, w_, nt) in tl:
                    c = b["c0"] + cl
                    g = (((r * NCHUNK + c) * NWIN + w_) * 2 + cls)
                    s0, s1 = starts[g], starts[g + 1]
                    n_real = s1 - s0
                    cap = nt * 128
                    assert n_real <= cap, (r, c, w_, cls, n_real, cap)
                    pad = cap - n_real
                    icols.append(np.concatenate(
                        [srow_s[s0:s1], np.zeros(pad, np.int64)]))
                    ed_cols.append(np.concatenate(
                        [dst_s[s0:s1] - r * OWN, np.zeros(pad, np.int64)]))
                    seg_cols.append(np.concatenate(
                        [(dst_s[s0:s1] - r * OWN - c * 128 - w_ * WINW).astype(np.float64),
                         np.full(pad, PAD_SEG)]))
        ilo = np.concatenate(ilo_cols).astype(np.int16)
        ihi = np.concatenate(ihi_cols).astype(np.int16)
        ed = np.concatenate(ed_cols).astype(np.int16)
        sg = np.concatenate(seg_cols).reshape(-1, 128).T.astype(bf16)
        per_core.append(dict(idxlo=_wrap_idxs(ilo), idxhi=_wrap_idxs(ihi),
                             idxe=_wrap_idxs(ed),
                             seg=np.ascontiguousarray(sg)))
    return blocks, per_core


def _build_program(blocks, heads_cfg):
    """Build the full 8-core SPMD Bass program. heads_cfg describes layers."""
    nc = bacc.Bacc("TRN2", target_bir_lowering=False, debug=False, num_devices=W,
                   num_swdge_queues=4)
    rg = [list(range(W))]

    NLO = sum(b["Tlo"] for b in blocks) * 128
    NHI = sum(b["Thi"] for b in blocks) * 128
    NT = sum(b["Tall"] for b in blocks)

    # ---------------- inputs ----------------
    pc0_shapes = heads_cfg["pc_shapes"]
    x1T_d = nc.dram_tensor("x1T", [128, OWN], dt.float32, kind="ExternalInput")
    idxlo_d = nc.dram_tensor("idxlo", list(pc0_shapes["idxlo"]), dt.int16, kind="ExternalInput")
    idxhi_d = nc.dram_tensor("idxhi", list(pc0_shapes["idxhi"]), dt.int16, kind="ExternalInput")
    idxe_d = nc.dram_tensor("idxe", list(pc0_shapes["idxe"]), dt.int16, kind="ExternalInput")
    seg_d = nc.dram_tensor("seg", list(pc0_shapes["seg"]), dt.bfloat16, kind="ExternalInput")
    J_d = nc.dram_tensor("J64", [128, WINW], dt.bfloat16, kind="ExternalInput")
    W1e_d = nc.dram_tensor("W1e", [128, PAY12], dt.float32, kind="ExternalInput")
    W2e_d = nc.dram_tensor("W2e", [2, 128, PAY12], dt.float32, kind="ExternalInput")
    W3e_d = nc.dram_tensor("W3e", [2, 128, PAY3], dt.float32, kind="ExternalInput")
    fcW_d = nc.dram_tensor("fcW", [64, N_CLASSES], dt.float32, kind="ExternalInput")
    b1_d = nc.dram_tensor("b1bc", [128, 256], dt.float32, kind="ExternalInput")
    b2_d = nc.dram_tensor("b2bc", [128, 256], dt.float32, kind="ExternalInput")
    b3_d = nc.dram_tensor("b3bc", [128, 64], dt.float32, kind="ExternalInput")
    fcb_d = nc.dram_tensor("fcbbc", [128, N_CLASSES], dt.float32, kind="ExternalInput")
    out_d = nc.dram_tensor("OUT", [OWN, N_CLASSES], dt.float32, kind="ExternalOutput")

    # ---------------- internals ----------------
    tables_lo = [
        nc.dram_tensor("table1lo", [NLOROW, ROW12], dt.bfloat16, kind="Internal", addr_space="Shared"),
        nc.dram_tensor("table2lo", [NLOROW, ROW12], dt.bfloat16, kind="Internal", addr_space="Shared"),
        nc.dram_tensor("table3lo", [NLOROW, ROW3], dt.bfloat16, kind="Internal", addr_space="Shared"),
    ]
    tables_hi = [
        nc.dram_tensor("table1hi", [NHIROW, ROW12], dt.bfloat16, kind="Internal", addr_space="Shared"),
        nc.dram_tensor("table2hi", [NHIROW, ROW12], dt.bfloat16, kind="Internal", addr_space="Shared"),
        nc.dram_tensor("table3hi", [NHIROW, ROW3], dt.bfloat16, kind="Internal", addr_space="Shared"),
    ]
    ag_ins = [
        nc.dram_tensor("agin1", [OWN, ROW12], dt.bfloat16, kind="Internal"),
        nc.dram_tensor("agin2", [OWN, ROW12], dt.bfloat16, kind="Internal"),
        nc.dram_tensor("agin3", [OWN, ROW3], dt.bfloat16, kind="Internal"),
    ]
    edsts = [
        nc.dram_tensor("edst1", [OWN, EROW], dt.bfloat16, kind="Internal"),
        nc.dram_tensor("edst2", [OWN, EROW], dt.bfloat16, kind="Internal"),
        nc.dram_tensor("edst3", [OWN, EROW], dt.bfloat16, kind="Internal"),
    ]
    xs = [
        None,
        nc.dram_tensor("x2", [OWN, 256], dt.float32, kind="Internal"),
        nc.dram_tensor("x3", [OWN, 256], dt.float32, kind="Internal"),
        nc.dram_tensor("x4", [OWN, 128], dt.float32, kind="Internal"),
    ]

    LAYERS = [
        dict(h=H1, F=256, row=ROW12, pay=PAY12, tlo=tables_lo[0], thi=tables_hi[0],
             agin=ag_ins[0], edst=edsts[0], b=b1_d, xout=xs[1], We=W1e_d, nkb=1),
        dict(h=H2, F=256, row=ROW12, pay=PAY12, tlo=tables_lo[1], thi=tables_hi[1],
             agin=ag_ins[1], edst=edsts[1], b=b2_d, xout=xs[2], We=W2e_d, nkb=2),
        dict(h=H3, F=64, row=ROW3, pay=PAY3, tlo=tables_lo[2], thi=tables_hi[2],
             agin=ag_ins[2], edst=edsts[2], b=b3_d, xout=xs[3], We=W3e_d, nkb=2),
    ]

    with tile.TileContext(nc) as tc:
        with tc.tile_pool(name="const", bufs=1) as cpool, \
             tc.tile_pool(name="np_sb", bufs=3) as npool, \
             tc.tile_pool(name="eg", bufs=2) as gpool, \
             tc.tile_pool(name="ep", bufs=3) as epool, \
             tc.tile_pool(name="e3", bufs=1) as e3pool, \
             tc.tile_pool(name="psum", bufs=2, space="PSUM") as pspool, \
             tc.tile_pool(name="psum_e", bufs=4, space="PSUM") as pspool_e:

            J_t = cpool.tile([128, WINW], dt.bfloat16)
            nc.sync.dma_start(out=J_t[:], in_=J_d.ap())
            # per-edge index / seg streams (identical for all three layers)
            idxlo_t = cpool.tile(list(pc0_shapes["idxlo"]), dt.int16)
            nc.sync.dma_start(out=idxlo_t[:], in_=idxlo_d.ap())
            idxhi_t = cpool.tile(list(pc0_shapes["idxhi"]), dt.int16)
            nc.sync.dma_start(out=idxhi_t[:], in_=idxhi_d.ap())
            idxe_t = cpool.tile(list(pc0_shapes["idxe"]), dt.int16)
            nc.sync.dma_start(out=idxe_t[:], in_=idxe_d.ap())
            seg_t = cpool.tile(list(pc0_shapes["seg"]), dt.bfloat16)
            nc.sync.dma_start(out=seg_t[:], in_=seg_d.ap())

            W1e_t = cpool.tile([128, PAY12], dt.float32)
            nc.sync.dma_start(out=W1e_t[:].bitcast(dt.float32r),
                              in_=W1e_d.ap().bitcast(dt.float32r))
            W2e_t = cpool.tile([128, 2 * PAY12], dt.float32)
            for kb in range(2):
                nc.sync.dma_start(out=W2e_t[:, kb * PAY12:(kb + 1) * PAY12].bitcast(dt.float32r),
                                  in_=W2e_d.ap()[kb].bitcast(dt.float32r))
            W3e_t = cpool.tile([128, 2 * PAY3], dt.float32)
            for kb in range(2):
                nc.sync.dma_start(out=W3e_t[:, kb * PAY3:(kb + 1) * PAY3].bitcast(dt.float32r),
                                  in_=W3e_d.ap()[kb].bitcast(dt.float32r))
            fcW_t = cpool.tile([64, N_CLASSES], dt.float32)
            nc.sync.dma_start(out=fcW_t[:], in_=fcW_d.ap())
            from concourse.masks import make_identity
            ident_t = cpool.tile([128, 128], dt.float32)
            make_identity(nc, ident_t[:])
            b_ts = {}
            for nm, d_, wdt in (("b1", b1_d, 256), ("b2", b2_d, 256),
                                ("b3", b3_d, 64), ("fcb", fcb_d, N_CLASSES)):
                t = cpool.tile([128, wdt], dt.float32, tag=f"bias_{nm}")
                nc.sync.dma_start(out=t[:], in_=d_.ap())
                b_ts[nm] = t

            def node_chunk(L, li, c, dep=None):
                """x @ [W|Wa_src|Wa_dst] for one 128-node chunk -> agin + edst
                rows.  Returns (agin_dma, edst_dma)."""
                F, pay, row = L["F"], L["pay"], L["row"]
                nh = L["h"]
                f32r = dt.float32r
                ps = pspool.tile([128, pay], dt.float32, tag="np_ps")
                if li == 0:
                    lhs = npool.tile([128, 128], dt.float32, tag="np_lhs")
                    nc.sync.dma_start(
                        out=lhs[:].bitcast(f32r),
                        in_=x1T_d.ap()[:, c * 128:(c + 1) * 128].bitcast(f32r))
                    nc.tensor.matmul(out=ps[:], lhsT=lhs[:].bitcast(f32r),
                                     rhs=W1e_t[:].bitcast(f32r),
                                     start=True, stop=True)
                else:
                    xin = xs[li]  # previous layer output [OWN, 256] f32
                    Wt = W2e_t if li == 1 else W3e_t
                    xc = npool.tile([128, 256], dt.float32, tag="np_xc")
                    rd = nc.sync.dma_start(
                        out=xc[:], in_=xin.ap()[c * 128:(c + 1) * 128, :])
                    if dep is not None:
                        _add_dep_helper(rd.ins, dep.ins, sync=True)
                    for kb in range(2):
                        pst = pspool.tile([128, 128], dt.float32, tag="np_tr")
                        nc.tensor.transpose(out=pst[:],
                                            in_=xc[:, kb * 128:(kb + 1) * 128],
                                            identity=ident_t[:])
                        lhs = npool.tile([128, 128], dt.float32, tag="np_lhs")
                        nc.vector.tensor_copy(out=lhs[:].bitcast(f32r), in_=pst[:])
                        nc.tensor.matmul(out=ps[:], lhsT=lhs[:].bitcast(f32r),
                                         rhs=Wt[:, kb * pay:(kb + 1) * pay].bitcast(f32r),
                                         start=(kb == 0), stop=(kb == 1))
                # epilogue: pack row_sb = [h bf16 | e_src f32] ; edst rows
                row_sb = npool.tile([128, row], dt.bfloat16, tag="np_row")
                nc.vector.tensor_copy(out=row_sb[:, 0:F], in_=ps[:, 0:F])
                rf32 = row_sb[:].bitcast(dt.float32)
                nc.vector.tensor_copy(out=rf32[:, F // 2:F // 2 + nh],
                                      in_=ps[:, F:F + nh])
                ed_sb = npool.tile([128, EROW], dt.bfloat16, tag="np_ed")
                ef32 = ed_sb[:].bitcast(dt.float32)
                nc.vector.tensor_copy(out=ef32[:, 0:nh],
                                      in_=ps[:, F + nh:F + 2 * nh])
                agd = nc.sync.dma_start(
                    out=L["agin"].ap()[c * 128:(c + 1) * 128, :], in_=row_sb[:])
                edd = nc.sync.dma_start(
                    out=L["edst"].ap()[c * 128:(c + 1) * 128, :], in_=ed_sb[:])
                return agd, edd

            def fc_chunk(c, dep):
                xc4 = npool.tile([128, 128], dt.float32, tag="fc_xc")
                rd = nc.sync.dma_start(
                    out=xc4[:], in_=xs[3].ap()[c * 128:(c + 1) * 128, 0:128])
                _add_dep_helper(rd.ins, dep.ins, sync=True)
                pst4 = pspool.tile([128, 128], dt.float32, tag="np_tr")
                nc.tensor.transpose(out=pst4[:], in_=xc4[:], identity=ident_t[:])
                lhs = npool.tile([128, 128], dt.float32, tag="fc_lhs")
                nc.scalar.activation(out=lhs[:], in_=pst4[:], func=AF.Copy)
                ps = pspool.tile([128, N_CLASSES], dt.float32, tag="np_ps")
                nc.tensor.matmul(out=ps[:], lhsT=lhs[0:64, :], rhs=fcW_t[:],
                                 start=True, stop=True)
                o_sb = npool.tile([128, N_CLASSES], dt.float32, tag="fc_o")
                nc.vector.tensor_tensor(out=o_sb[:], in0=ps[:],
                                        in1=b_ts["fcb"][:], op=OP.add)
                nc.sync.dma_start(out=out_d.ap()[c * 128:(c + 1) * 128, :],
                                  in_=o_sb[:])

            def edge_phase(L, li, after_chunk):
                """gather + attention + segment-reduce; writes L["xout"].
                after_chunk(c, xw_dma) is invoked as each chunk's output DMA is
                emitted, so the next layer's node work interleaves in program
                order with this layer's edge blocks."""
                F, pay, row, nh = L["F"], L["pay"], L["row"], L["h"]
                eds = L["edst"]
                glo_insts, ghi_insts, ge_insts = [], [], []
                done_q = []   # (chunk, xw_dma) awaiting the lagged after_chunk
                LAG = 2       # blocks of delay so deps are met when emitted
                lo_view = L["tlo"].ap()[:, 0:pay]
                hi_view = L["thi"].ap()[:, 0:pay]
                ed_view = eds.ap()[:, 0:8]
                toff = offlo = offhi = 0
                for bi, b in enumerate(blocks):
                    T, Tlo, Thi = b["Tall"], b["Tlo"], b["Thi"]
                    nb = b["nb"]
                    while len(done_q) > LAG * CPB:
                        after_chunk(*done_q.pop(0))
                    G_t = gpool.tile([128, T, pay], dt.bfloat16, tag="G")
                    S_t = gpool.tile([128, T, WINW], dt.bfloat16, tag="S")

                    # four queue-balanced gathers per block: each SWDGE queue
                    # is served by its own Q7 core pair, so spreading calls
                    # round-robin parallelizes descriptor generation 4x
                    q0 = bi % 4
                    if Tlo:
                        glo_insts.append(nc.gpsimd.dma_gather(
                            G_t[:, 0:Tlo, :], lo_view,
                            idxlo_t[:, offlo // 16:(offlo + Tlo * 128) // 16],
                            Tlo * 128, Tlo * 128, pay, elem_step=row,
                            single_packet=False, queue_num=q0))
                    if Thi:
                        ghi_insts.append(nc.gpsimd.dma_gather(
                            G_t[:, Tlo:T, :], hi_view,
                            idxhi_t[:, offhi // 16:(offhi + Thi * 128) // 16],
                            Thi * 128, Thi * 128, pay, elem_step=row,
                            single_packet=False, queue_num=(q0 + 1) % 4))
                    if nh > 1:
                        E_t = gpool.tile([128, T, 8], dt.bfloat16, tag="E")
                        if Tlo:
                            ge_insts.append(nc.gpsimd.dma_gather(
                                E_t[:, 0:Tlo, :], ed_view,
                                idxe_t[:, toff * 8:(toff + Tlo) * 8],
                                Tlo * 128, Tlo * 128, 8, elem_step=EROW,
                                single_packet=False, queue_num=(q0 + 2) % 4))
                        if Thi:
                            ge_insts.append(nc.gpsimd.dma_gather(
                                E_t[:, Tlo:T, :], ed_view,
                                idxe_t[:, (toff + Tlo) * 8:(toff + T) * 8],
                                Thi * 128, Thi * 128, 8, elem_step=EROW,
                                single_packet=False, queue_num=(q0 + 3) % 4))
                    else:
                        # single-head layer: replicate the block's e_dst rows to
                        # all partitions once (128 fat descriptors) instead of a
                        # per-edge gather — the per-edge selection reuses S
                        EWb = e3pool.tile([128, nb * 128, 2], dt.bfloat16, tag="EWb")
                        eap = eds.ap()
                        src_ap = bass.AP(eap.tensor,
                                         eap.offset + b["c0"] * 128 * EROW,
                                         [[0, 128], [EROW, nb * 128], [1, 2]])
                        ge_insts.append(nc.sync.dma_start(out=EWb[:], in_=src_ap))

                    consumers = []
                    # S build: S[p,t,j] = (seg[p,t] == j)
                    in0 = seg_t[:, toff:toff + T].to_broadcast([128, T, WINW])
                    jap = J_t[:]
                    in1 = bass.AP(jap.tensor, jap.offset,
                                  [jap.ap[0], [0, T], [1, WINW]])
                    nc.vector.tensor_tensor(out=S_t[:], in0=in0, in1=in1,
                                            op=OP.is_equal)

                    # edge weights x = exp(leaky(e_src + e_dst)); leaky+exp on
                    # the (otherwise idle) scalar engine
                    gf32 = G_t[:].bitcast(dt.float32)   # [128, T, pay//2]
                    z_t = gpool.tile([128, T, nh], dt.float32, tag="z")
                    if nh > 1:
                        ef32 = E_t[:].bitcast(dt.float32)   # [128, T, 4]
                        consumers.append(nc.vector.tensor_tensor(
                            out=z_t[:], in0=gf32[:, :, F // 2:F // 2 + nh],
                            in1=ef32[:, :, 0:nh], op=OP.add))
                    else:
                        # per-edge e_dst = sum_j S[p,t,j] * EWb[window j] via a
                        # masked multiply + innermost reduce
                        tmp3 = e3pool.tile([128, T, WINW], dt.bfloat16, tag="tmp3")
                        ewf = EWb[:].bitcast(dt.float32)    # [128, nb*128, 1]
                        t0 = 0
                        for (cl, w_, nt) in b["lo"] + b["hi"]:
                            in1 = bass.AP(ewf.tensor,
                                          ewf.offset + cl * 128 + w_ * WINW,
                                          [ewf.ap[0], [0, nt], [1, WINW]])
                            nc.vector.tensor_tensor(
                                out=tmp3[:, t0:t0 + nt, :],
                                in0=S_t[:, t0:t0 + nt, :], in1=in1, op=OP.mult)
                            t0 += nt
                        E_f = e3pool.tile([128, T, 1], dt.float32, tag="Ef")
                        nc.vector.tensor_reduce(out=E_f[:], in_=tmp3[:],
                                                axis=mybir.AxisListType.X,
                                                op=OP.add)
                        consumers.append(nc.vector.tensor_tensor(
                            out=z_t[:], in0=gf32[:, :, F // 2:F // 2 + nh],
                            in1=E_f[:], op=OP.add))
                    nc.vector.scalar_tensor_tensor(
                        out=z_t[:], in0=z_t[:], scalar=NEG_SLOPE, in1=z_t[:],
                        op0=OP.mult, op1=OP.max)
                    # x broadcast-expanded to per-channel lanes so the DVE fold
                    # reads a contiguous operand (2x bf16 rate)
                    x_e = gpool.tile([128, T, nh, CH], dt.bfloat16, tag="xe")
                    zb = bass.AP(z_t[:].tensor, z_t[:].offset,
                                 [z_t[:].ap[0], [nh, T], [1, nh], [0, CH]])
                    nc.scalar.activation(out=x_e[:], in_=zb, func=AF.Exp)
                    consumers.append(nc.scalar.activation(
                        out=G_t[:, :, F:F + nh], in_=z_t[:], func=AF.Exp))

                    # fold x into G (in place)
                    g4 = bass.AP(G_t[:].tensor, G_t[:].offset,
                                 [G_t[:].ap[0], [pay, T], [CH, nh], [1, CH]])
                    consumers.append(nc.vector.tensor_tensor(out=g4, in0=g4,
                                                             in1=x_e[:], op=OP.mult))

                    # matmuls: per chunk psum [128, F+nh]
                    pss = []
                    for cl in range(nb):
                        ep_ps = pspool_e.tile([128, F + nh], dt.float32, tag="ep_ps")
                        pss.append(ep_ps)
                    # tile sequence: lo tiles then hi tiles; stop flag on the
                    # last tile of each (cl, w) across both halves
                    seq = []
                    for tl in (b["lo"], b["hi"]):
                        for (cl, w_, nt) in tl:
                            for k in range(nt):
                                seq.append((cl, w_))
                    last_of = {}
                    for i, kw in enumerate(seq):
                        last_of[kw] = i
                    started = {}
                    for t_id, (cl, w_) in enumerate(seq):
                        keyw = (cl, w_)
                        first = keyw not in started
                        started[keyw] = True
                        consumers.append(nc.tensor.matmul(
                            out=pss[cl][w_ * WINW:(w_ + 1) * WINW, :],
                            lhsT=S_t[:, t_id, :],
                            rhs=G_t[:, t_id, 0:F + nh],
                            start=first, stop=(last_of[keyw] == t_id),
                            tile_position=(0, w_ * WINW),
                            skip_group_check=True))
                    # epilogue per chunk
                    for cl in range(nb):
                        c = b["c0"] + cl
                        ps = pss[cl]
                        inv = epool.tile([128, nh], dt.float32, tag="inv")
                        nc.vector.tensor_scalar_add(out=inv[:], in0=ps[:, F:F + nh],
                                                    scalar1=1e-20)
                        nc.vector.reciprocal(out=inv[:], in_=inv[:])
                        if li < 2:
                            o_sb = epool.tile([128, 256], dt.float32, tag="o_sb")
                        else:
                            o_sb = epool.tile([128, 128], dt.float32, tag="o_sb3")
                            nc.vector.memset(o_sb[:, 64:128], 0.0)
                        for h_ in range(nh):
                            nc.vector.scalar_tensor_tensor(
                                out=o_sb[:, h_ * CH:(h_ + 1) * CH],
                                in0=ps[:, h_ * CH:(h_ + 1) * CH],
                                scalar=inv[:, h_:h_ + 1],
                                in1=b_ts["b" + str(li + 1)][:, h_ * CH:(h_ + 1) * CH],
                                op0=OP.mult, op1=OP.add)
                        nc.scalar.activation(out=o_sb[:, 0:F], in_=o_sb[:, 0:F],
                                             func=AF.Relu)
                        xwd = nc.sync.dma_start(
                            out=L["xout"].ap()[c * 128:(c + 1) * 128, :],
                            in_=o_sb[:])
                        done_q.append((c, xwd))
                    toff += T; offlo += Tlo * 128; offhi += Thi * 128
                for ent in done_q:
                    after_chunk(*ent)
                return glo_insts, ghi_insts, ge_insts

            NLOCH = LOH // 128   # 25 chunks in the lo half
            ag_st = {0: [], 1: [], 2: []}
            ed_st = {0: [], 1: [], 2: []}
            ccs = {}

            def emit_ccs_maybe(li):
                # two half-table AllGathers per layer: lo fires as soon as the
                # first 25 node chunks are written (mid previous edge phase)
                L = LAYERS[li]
                if len(ag_st[li]) == NLOCH and (li, 1) not in ccs:
                    cc1 = nc.gpsimd.collective_compute(
                        "AllGather", OP.bypass, replica_groups=rg,
                        ins=[L["agin"].ap()[0:LOH, :]], outs=[L["tlo"].ap()])
                    for wdm in ag_st[li]:
                        _add_dep_helper(cc1.ins, wdm.ins, sync=True)
                    ccs[(li, 1)] = cc1
                if len(ag_st[li]) == NCHUNK and (li, 2) not in ccs:
                    cc2 = nc.gpsimd.collective_compute(
                        "AllGather", OP.bypass, replica_groups=rg,
                        ins=[L["agin"].ap()[LOH:OWN, :]], outs=[L["thi"].ap()])
                    for wdm in ag_st[li][NLOCH:]:
                        _add_dep_helper(cc2.ins, wdm.ins, sync=True)
                    ccs[(li, 2)] = cc2

            def after_chunk(li, c, xwd):
                # edge phase of layer li just produced chunk c of xs[li+1]
                if li < 2:
                    agd, edd = node_chunk(LAYERS[li + 1], li + 1, c, dep=xwd)
                    ag_st[li + 1].append(agd)
                    ed_st[li + 1].append(edd)
                    emit_ccs_maybe(li + 1)
                else:
                    fc_chunk(c, xwd)

            # layer-1 node phase upfront (reads only the x1T input)
            for c in range(NCHUNK):
                agd, edd = node_chunk(LAYERS[0], 0, c)
                ag_st[0].append(agd)
                ed_st[0].append(edd)
                emit_ccs_maybe(0)

            for li, L in enumerate(LAYERS):
                glo, ghi, ge = edge_phase(
                    L, li, lambda c, xwd, li=li: after_chunk(li, c, xwd))
                for gi in glo:
                    _add_dep_helper(gi.ins, ccs[(li, 1)].ins, sync=True)
                for gi in ghi:
                    _add_dep_helper(gi.ins, ccs[(li, 2)].ins, sync=True)
                for gi in ge:
                    for wdm in ed_st[li]:
                        _add_dep_helper(gi.ins, wdm.ins, sync=True)

    nc.compile()
    return nc


def kernel(x, edge_index, W1, a1_src, a1_dst, b1, W2, a2_src, a2_dst, b2,
           W3, a3_src, a3_dst, b3, fc_W, fc_b):
    x = np.asarray(x, np.float32)
    edge_index = np.asarray(edge_index)
    blocks, per_core = _host_prep(edge_index)

    def ext(Wm, a_s, a_d, nh):
        Wm = np.asarray(Wm, np.float32)
        F = Wm.shape[1]
        A_s = np.zeros((F, nh), np.float32)
        A_d = np.zeros((F, nh), np.float32)
        for h_ in range(nh):
            A_s[h_ * CH:(h_ + 1) * CH, h_] = np.asarray(a_s, np.float32)[h_]
            A_d[h_ * CH:(h_ + 1) * CH, h_] = np.asarray(a_d, np.float32)[h_]
        return np.concatenate([Wm, Wm @ A_s, Wm @ A_d], axis=1)

    W1e = ext(W1, a1_src, a1_dst, H1)                          # [128, 264]
    W2e = ext(W2, a2_src, a2_dst, H2).reshape(2, 128, PAY12)
    W3e = ext(W3, a3_src, a3_dst, H3).reshape(2, 128, PAY3)
    fcWb = np.asarray(fc_W, np.float32)
    J64 = np.broadcast_to(np.arange(WINW, dtype=np.float32), (128, WINW)).astype(bf16)

    def bc(v, w_):
        return np.broadcast_to(np.asarray(v, np.float32)[None, :], (128, w_)).copy()

    xpad = np.zeros((NPAD, IN_DIM), np.float32)
    xpad[:N_NODES] = x

    heads_cfg = dict(pc_shapes={k: per_core[0][k].shape for k in
                                ("idxlo", "idxhi", "idxe", "seg")})
    nc = _build_program(blocks, heads_cfg)

    in_maps = []
    for r in range(W):
        pc = per_core[r]
        in_maps.append({
            "x1T": np.ascontiguousarray(xpad[r * OWN:(r + 1) * OWN].T),
            "idxlo": pc["idxlo"], "idxhi": pc["idxhi"], "idxe": pc["idxe"],
            "seg": pc["seg"],
            "J64": J64, "W1e": W1e, "W2e": W2e, "W3e": W3e, "fcW": fcWb,
            "b1bc": bc(b1, 256), "b2bc": bc(b2, 256), "b3bc": bc(b3, 64),
            "fcbbc": bc(fc_b, N_CLASSES),
        })

    res = bass_utils.run_bass_kernel_spmd(nc, in_maps, core_ids=list(range(W)))
    global LAST_RES
    LAST_RES = res
    out = np.concatenate([res.results[r]["OUT"] for r in range(W)], axis=0)
    return out[:N_NODES].astype(np.float32)


# revision 39
# speedup vs baseline: 1.0411x; 1.0411x over previous
"""3-layer GAT (GATConv x3 + FC) on 8 Trainium2 NeuronCores.

Strategy: dst-sorted edge partitioning (each core owns a contiguous node range
and all edges into it), per-layer node-parallel feature matmul + AllGather of a
gatherable node table [h | e_src], then an edge phase per core: batched
dma_gather of src rows (int16 indices; the table is split into lo/hi halves so
indices fit in int16, and edges are grouped per (chunk, window, half)),
exp(leaky(e_src+e_dst)) edge weights, and segment reduction via one-hot
selection-matrix matmuls accumulated in PSUM per 128-node chunk.  Softmax is
unnormalized (exp without max subtraction) with per-node post-normalization by
the gathered weight sum; self-loops guarantee the sum stays well away from 0.
"""
import os, sys
sys.path.insert(0, '/opt/trn_rl_repo')
import math
import numpy as np
ATH_STAGE = int(os.environ.get("ATH_STAGE", "6"))
ATH_EDGE = int(os.environ.get("ATH_EDGE", "4"))
import ml_dtypes

import concourse.bass as bass
import concourse.bacc as bacc
import concourse.mybir as mybir
import concourse.tile as tile
from concourse import bass_utils
from concourse.bass import _add_dep_helper

# ---- model constants (must match reference.py) ----
NEG_SLOPE = 0.2
H1, H2, H3 = 4, 4, 1
CH = 64
N_NODES = 50000
N_EDGES = 800000
IN_DIM = 128
N_CLASSES = 10

W = 8                    # cores
OWN = 6272               # nodes per core (49 chunks of 128)
NPAD = W * OWN           # 50176
LOH = 3200               # within-core lo-half rows (25 chunks)
NLOROW = W * LOH         # 25600 rows in the lo table (< int16 max)
NHIROW = W * (OWN - LOH) # 24576 rows in the hi table
NCHUNK = OWN // 128      # 49
CPB = 3                  # chunks per gather block
NWIN = 2                 # 64-node windows per chunk
WINW = 64
PAD_SEG = 99.0

# table row layout (bf16 slots)
ROW12 = 384              # layers 1/2: [h(256) | e_src f32 (8 slots) | pad] = 768B stride
PAY12 = 264
ROW3 = 128               # layer 3: [h(64) | e_src f32 (2 slots) | pad] = 256B stride
PAY3 = 66
EROW = 128               # e_dst local table row: 256B stride, leading 8 (or 2) slots = f32

dt = mybir.dt
AF = mybir.ActivationFunctionType
OP = mybir.AluOpType
bf16 = ml_dtypes.bfloat16

# relax dma_gather's elem_size%256 assert (q7 ucode only requires the row
# stride to be a multiple of 256B; elem_size is the per-descriptor length)
import inspect as _inspect, textwrap as _textwrap
_src = _textwrap.dedent(_inspect.getsource(bass.BassGpSimd.dma_gather))
_src = _src.replace("elem_size_bytes > 0 and elem_size_bytes % 256 == 0",
                    "elem_size_bytes > 0")
_ns = dict(bass.__dict__)
exec(compile(_src, "<dma_gather_patched>", "exec"), _ns)
bass.BassGpSimd.dma_gather = _ns["dma_gather"]


def _wrap_idxs(idx_i16):
    """dma_gather index layout: idx i at [i%16, i//16], replicated to 128 parts."""
    n = idx_i16.shape[0]
    cols = n // 16
    arr = idx_i16.reshape(cols, 16).T.copy()
    return np.tile(arr, (8, 1))


def _host_prep(edge_index):
    """Edge sorting/tiling; returns per-core input arrays + static tile metadata.

    Edges (with self-loops) are grouped per (core, chunk, window, srchalf) and
    padded to 128-edge tiles; tile counts are the max over cores so the SPMD
    program is identical everywhere.  Per block (CPB chunks) the tile order is
    all lo-half tiles (groups in order) then all hi-half tiles.
    """
    src = np.concatenate([edge_index[0].astype(np.int64),
                          np.arange(N_NODES, dtype=np.int64)])
    dst = np.concatenate([edge_index[1].astype(np.int64),
                          np.arange(N_NODES, dtype=np.int64)])

    core = dst // OWN
    chunk = (dst % OWN) // 128
    win = ((dst % OWN) % 128) // WINW
    # lo/hi = within-core row halves (so each half AllGathers separately and
    # each table stays under the int16 index limit)
    half = ((src % OWN) >= LOH).astype(np.int64)
    srow = np.where(half == 0,
                    (src // OWN) * LOH + (src % OWN),
                    (src // OWN) * (OWN - LOH) + (src % OWN) - LOH)

    key = (((core * NCHUNK + chunk) * NWIN + win) * 2 + half)
    korder = np.argsort(key, kind="stable")
    src_s, dst_s, srow_s = src[korder], dst[korder], srow[korder]
    ngroups = W * NCHUNK * NWIN * 2
    counts = np.bincount(key[korder], minlength=ngroups).reshape(
        W, NCHUNK, NWIN, 2)
    starts = np.zeros(ngroups + 1, dtype=np.int64)
    np.cumsum(counts.reshape(-1), out=starts[1:])

    tiles_per = np.ceil(counts / 128.0).astype(np.int64).max(axis=0)  # [NCHUNK][NWIN][2]

    blocks = []
    c0 = 0
    while c0 < NCHUNK:
        nb = min(CPB, NCHUNK - c0)
        lo, hi = [], []
        for cl in range(nb):
            for w_ in range(NWIN):
                ntl = int(tiles_per[c0 + cl][w_][0])
                nth = int(tiles_per[c0 + cl][w_][1])
                if ntl:
                    lo.append((cl, w_, ntl))
                if nth:
                    hi.append((cl, w_, nth))
        Tlo = sum(t[2] for t in lo)
        Thi = sum(t[2] for t in hi)
        blocks.append(dict(c0=c0, nb=nb, lo=lo, hi=hi,
                           Tlo=Tlo, Thi=Thi, Tall=Tlo + Thi))
        c0 += nb

    per_core = []
    for r in range(W):
        ilo_cols, ihi_cols, ed_cols, seg_cols = [], [], [], []
        for b in blocks:
            for cls, tl, icols in ((0, b["lo"], ilo_cols), (1, b["hi"], ihi_cols)):
                for (cl, w_, nt) in tl:
                    c = b["c0"] + cl
                    g = (((r * NCHUNK + c) * NWIN + w_) * 2 + cls)
                    s0, s1 = starts[g], starts[g + 1]
                    n_real = s1 - s0
                    cap = nt * 128
                    assert n_real <= cap, (r, c, w_, cls, n_real, cap)
                    pad = cap - n_real
                    icols.append(np.concatenate(
                        [srow_s[s0:s1], np.zeros(pad, np.int64)]))
                    ed_cols.append(np.concatenate(
                        [dst_s[s0:s1] - r * OWN, np.zeros(pad, np.int64)]))
                    seg_cols.append(np.concatenate(
                        [(dst_s[s0:s1] - r * OWN - c * 128 - w_ * WINW).astype(np.float64),
                         np.full(pad, PAD_SEG)]))
        ilo = np.concatenate(ilo_cols).astype(np.int16)
        ihi = np.concatenate(ihi_cols).astype(np.int16)
        ed = np.concatenate(ed_cols).astype(np.int16)
        sg = np.concatenate(seg_cols).reshape(-1, 128).T.astype(bf16)
        per_core.append(dict(idxlo=_wrap_idxs(ilo), idxhi=_wrap_idxs(ihi),
                             idxe=_wrap_idxs(ed),
                             seg=np.ascontiguousarray(sg)))
    return blocks, per_core


def _build_program(blocks, heads_cfg):
    """Build the full 8-core SPMD Bass program. heads_cfg describes layers."""
    nc = bacc.Bacc("TRN2", target_bir_lowering=False, debug=False, num_devices=W,
                   num_swdge_queues=4)
    rg = [list(range(W))]

    NLO = sum(b["Tlo"] for b in blocks) * 128
    NHI = sum(b["Thi"] for b in blocks) * 128
    NT = sum(b["Tall"] for b in blocks)

    # ---------------- inputs ----------------
    pc0_shapes = heads_cfg["pc_shapes"]
    x1T_d = nc.dram_tensor("x1T", [128, OWN], dt.float32, kind="ExternalInput")
    idxlo_d = nc.dram_tensor("idxlo", list(pc0_shapes["idxlo"]), dt.int16, kind="ExternalInput")
    idxhi_d = nc.dram_tensor("idxhi", list(pc0_shapes["idxhi"]), dt.int16, kind="ExternalInput")
    idxe_d = nc.dram_tensor("idxe", list(pc0_shapes["idxe"]), dt.int16, kind="ExternalInput")
    seg_d = nc.dram_tensor("seg", list(pc0_shapes["seg"]), dt.bfloat16, kind="ExternalInput")
    J_d = nc.dram_tensor("J64", [128, WINW], dt.bfloat16, kind="ExternalInput")
    W1e_d = nc.dram_tensor("W1e", [128, PAY12], dt.float32, kind="ExternalInput")
    W2e_d = nc.dram_tensor("W2e", [2, 128, PAY12], dt.float32, kind="ExternalInput")
    W3e_d = nc.dram_tensor("W3e", [2, 128, PAY3], dt.float32, kind="ExternalInput")
    fcW_d = nc.dram_tensor("fcW", [64, N_CLASSES], dt.float32, kind="ExternalInput")
    b1_d = nc.dram_tensor("b1bc", [128, 256], dt.float32, kind="ExternalInput")
    b2_d = nc.dram_tensor("b2bc", [128, 256], dt.float32, kind="ExternalInput")
    b3_d = nc.dram_tensor("b3bc", [128, 64], dt.float32, kind="ExternalInput")
    fcb_d = nc.dram_tensor("fcbbc", [128, N_CLASSES], dt.float32, kind="ExternalInput")
    out_d = nc.dram_tensor("OUT", [OWN, N_CLASSES], dt.float32, kind="ExternalOutput")

    # ---------------- internals ----------------
    tables_lo = [
        nc.dram_tensor("table1lo", [NLOROW, ROW12], dt.bfloat16, kind="Internal", addr_space="Shared"),
        nc.dram_tensor("table2lo", [NLOROW, ROW12], dt.bfloat16, kind="Internal", addr_space="Shared"),
        nc.dram_tensor("table3lo", [NLOROW, ROW3], dt.bfloat16, kind="Internal", addr_space="Shared"),
    ]
    tables_hi = [
        nc.dram_tensor("table1hi", [NHIROW, ROW12], dt.bfloat16, kind="Internal", addr_space="Shared"),
        nc.dram_tensor("table2hi", [NHIROW, ROW12], dt.bfloat16, kind="Internal", addr_space="Shared"),
        nc.dram_tensor("table3hi", [NHIROW, ROW3], dt.bfloat16, kind="Internal", addr_space="Shared"),
    ]
    ag_ins = [
        nc.dram_tensor("agin1", [OWN, ROW12], dt.bfloat16, kind="Internal"),
        nc.dram_tensor("agin2", [OWN, ROW12], dt.bfloat16, kind="Internal"),
        nc.dram_tensor("agin3", [OWN, ROW3], dt.bfloat16, kind="Internal"),
    ]
    edsts = [
        nc.dram_tensor("edst1", [OWN, EROW], dt.bfloat16, kind="Internal"),
        nc.dram_tensor("edst2", [OWN, EROW], dt.bfloat16, kind="Internal"),
        nc.dram_tensor("edst3", [OWN, EROW], dt.bfloat16, kind="Internal"),
    ]
    xs = [
        None,
        nc.dram_tensor("x2", [OWN, 256], dt.float32, kind="Internal"),
        nc.dram_tensor("x3", [OWN, 256], dt.float32, kind="Internal"),
        nc.dram_tensor("x4", [OWN, 128], dt.float32, kind="Internal"),
    ]

    LAYERS = [
        dict(h=H1, F=256, row=ROW12, pay=PAY12, tlo=tables_lo[0], thi=tables_hi[0],
             agin=ag_ins[0], edst=edsts[0], b=b1_d, xout=xs[1], We=W1e_d, nkb=1),
        dict(h=H2, F=256, row=ROW12, pay=PAY12, tlo=tables_lo[1], thi=tables_hi[1],
             agin=ag_ins[1], edst=edsts[1], b=b2_d, xout=xs[2], We=W2e_d, nkb=2),
        dict(h=H3, F=64, row=ROW3, pay=PAY3, tlo=tables_lo[2], thi=tables_hi[2],
             agin=ag_ins[2], edst=edsts[2], b=b3_d, xout=xs[3], We=W3e_d, nkb=2),
    ]

    with tile.TileContext(nc) as tc:
        with tc.tile_pool(name="const", bufs=1) as cpool, \
             tc.tile_pool(name="np_sb", bufs=3) as npool, \
             tc.tile_pool(name="eg", bufs=2) as gpool, \
             tc.tile_pool(name="ep", bufs=3) as epool, \
             tc.tile_pool(name="psum", bufs=2, space="PSUM") as pspool, \
             tc.tile_pool(name="psum_e", bufs=4, space="PSUM") as pspool_e:

            J_t = cpool.tile([128, WINW], dt.bfloat16)
            nc.sync.dma_start(out=J_t[:], in_=J_d.ap())
            # per-edge index / seg streams (identical for all three layers)
            idxlo_t = cpool.tile(list(pc0_shapes["idxlo"]), dt.int16)
            nc.sync.dma_start(out=idxlo_t[:], in_=idxlo_d.ap())
            idxhi_t = cpool.tile(list(pc0_shapes["idxhi"]), dt.int16)
            nc.sync.dma_start(out=idxhi_t[:], in_=idxhi_d.ap())
            idxe_t = cpool.tile(list(pc0_shapes["idxe"]), dt.int16)
            nc.sync.dma_start(out=idxe_t[:], in_=idxe_d.ap())
            seg_t = cpool.tile(list(pc0_shapes["seg"]), dt.bfloat16)
            nc.sync.dma_start(out=seg_t[:], in_=seg_d.ap())

            W1e_t = cpool.tile([128, PAY12], dt.float32)
            nc.sync.dma_start(out=W1e_t[:].bitcast(dt.float32r),
                              in_=W1e_d.ap().bitcast(dt.float32r))
            W2e_t = cpool.tile([128, 2 * PAY12], dt.float32)
            for kb in range(2):
                nc.sync.dma_start(out=W2e_t[:, kb * PAY12:(kb + 1) * PAY12].bitcast(dt.float32r),
                                  in_=W2e_d.ap()[kb].bitcast(dt.float32r))
            W3e_t = cpool.tile([128, 2 * PAY3], dt.float32)
            for kb in range(2):
                nc.sync.dma_start(out=W3e_t[:, kb * PAY3:(kb + 1) * PAY3].bitcast(dt.float32r),
                                  in_=W3e_d.ap()[kb].bitcast(dt.float32r))
            fcW_t = cpool.tile([64, N_CLASSES], dt.float32)
            nc.sync.dma_start(out=fcW_t[:], in_=fcW_d.ap())
            from concourse.masks import make_identity
            ident_t = cpool.tile([128, 128], dt.float32)
            make_identity(nc, ident_t[:])
            b_ts = {}
            for nm, d_, wdt in (("b1", b1_d, 256), ("b2", b2_d, 256),
                                ("b3", b3_d, 64), ("fcb", fcb_d, N_CLASSES)):
                t = cpool.tile([128, wdt], dt.float32, tag=f"bias_{nm}")
                nc.sync.dma_start(out=t[:], in_=d_.ap())
                b_ts[nm] = t

            def node_chunk(L, li, c, dep=None):
                """x @ [W|Wa_src|Wa_dst] for one 128-node chunk -> agin + edst
                rows.  Returns (agin_dma, edst_dma)."""
                F, pay, row = L["F"], L["pay"], L["row"]
                nh = L["h"]
                f32r = dt.float32r
                ps = pspool.tile([128, pay], dt.float32, tag="np_ps")
                if li == 0:
                    lhs = npool.tile([128, 128], dt.float32, tag="np_lhs")
                    nc.sync.dma_start(
                        out=lhs[:].bitcast(f32r),
                        in_=x1T_d.ap()[:, c * 128:(c + 1) * 128].bitcast(f32r))
                    nc.tensor.matmul(out=ps[:], lhsT=lhs[:].bitcast(f32r),
                                     rhs=W1e_t[:].bitcast(f32r),
                                     start=True, stop=True)
                else:
                    xin = xs[li]  # previous layer output [OWN, 256] f32
                    Wt = W2e_t if li == 1 else W3e_t
                    xc = npool.tile([128, 256], dt.float32, tag="np_xc")
                    rd = nc.sync.dma_start(
                        out=xc[:], in_=xin.ap()[c * 128:(c + 1) * 128, :])
                    if dep is not None:
                        _add_dep_helper(rd.ins, dep.ins, sync=True)
                    for kb in range(2):
                        pst = pspool.tile([128, 128], dt.float32, tag="np_tr")
                        nc.tensor.transpose(out=pst[:],
                                            in_=xc[:, kb * 128:(kb + 1) * 128],
                                            identity=ident_t[:])
                        lhs = npool.tile([128, 128], dt.float32, tag="np_lhs")
                        nc.vector.tensor_copy(out=lhs[:].bitcast(f32r), in_=pst[:])
                        nc.tensor.matmul(out=ps[:], lhsT=lhs[:].bitcast(f32r),
                                         rhs=Wt[:, kb * pay:(kb + 1) * pay].bitcast(f32r),
                                         start=(kb == 0), stop=(kb == 1))
                # epilogue: pack row_sb = [h bf16 | e_src f32] ; edst rows
                row_sb = npool.tile([128, row], dt.bfloat16, tag="np_row")
                nc.vector.tensor_copy(out=row_sb[:, 0:F], in_=ps[:, 0:F])
                rf32 = row_sb[:].bitcast(dt.float32)
                nc.vector.tensor_copy(out=rf32[:, F // 2:F // 2 + nh],
                                      in_=ps[:, F:F + nh])
                ed_sb = npool.tile([128, EROW], dt.bfloat16, tag="np_ed")
                ef32 = ed_sb[:].bitcast(dt.float32)
                nc.vector.tensor_copy(out=ef32[:, 0:nh],
                                      in_=ps[:, F + nh:F + 2 * nh])
                agd = nc.sync.dma_start(
                    out=L["agin"].ap()[c * 128:(c + 1) * 128, :], in_=row_sb[:])
                edd = nc.sync.dma_start(
                    out=L["edst"].ap()[c * 128:(c + 1) * 128, :], in_=ed_sb[:])
                return agd, edd

            def fc_chunk(c, dep):
                xc4 = npool.tile([128, 128], dt.float32, tag="fc_xc")
                rd = nc.sync.dma_start(
                    out=xc4[:], in_=xs[3].ap()[c * 128:(c + 1) * 128, 0:128])
                _add_dep_helper(rd.ins, dep.ins, sync=True)
                pst4 = pspool.tile([128, 128], dt.float32, tag="np_tr")
                nc.tensor.transpose(out=pst4[:], in_=xc4[:], identity=ident_t[:])
                lhs = npool.tile([128, 128], dt.float32, tag="fc_lhs")
                nc.scalar.activation(out=lhs[:], in_=pst4[:], func=AF.Copy)
                ps = pspool.tile([128, N_CLASSES], dt.float32, tag="np_ps")
                nc.tensor.matmul(out=ps[:], lhsT=lhs[0:64, :], rhs=fcW_t[:],
                                 start=True, stop=True)
                o_sb = npool.tile([128, N_CLASSES], dt.float32, tag="fc_o")
                nc.vector.tensor_tensor(out=o_sb[:], in0=ps[:],
                                        in1=b_ts["fcb"][:], op=OP.add)
                nc.sync.dma_start(out=out_d.ap()[c * 128:(c + 1) * 128, :],
                                  in_=o_sb[:])

            def edge_phase(L, li, after_chunk):
                """gather + attention + segment-reduce; writes L["xout"].
                after_chunk(c, xw_dma) is invoked as each chunk's output DMA is
                emitted, so the next layer's node work interleaves in program
                order with this layer's edge blocks."""
                F, pay, row, nh = L["F"], L["pay"], L["row"], L["h"]
                eds = L["edst"]
                glo_insts, ghi_insts, ge_insts = [], [], []
                done_q = []   # (chunk, xw_dma) awaiting the lagged after_chunk
                LAG = 2       # blocks of delay so deps are met when emitted
                lo_view = L["tlo"].ap()[:, 0:pay]
                hi_view = L["thi"].ap()[:, 0:pay]
                ed_view = eds.ap()[:, 0:8]
                toff = offlo = offhi = 0
                for bi, b in enumerate(blocks):
                    T, Tlo, Thi = b["Tall"], b["Tlo"], b["Thi"]
                    nb = b["nb"]
                    while len(done_q) > LAG * CPB:
                        after_chunk(*done_q.pop(0))
                    G_t = gpool.tile([128, T, pay], dt.bfloat16, tag="G")
                    E_t = gpool.tile([128, T, 8], dt.bfloat16, tag="E")
                    S_t = gpool.tile([128, T, WINW], dt.bfloat16, tag="S")

                    # four queue-balanced gathers per block: each SWDGE queue
                    # is served by its own Q7 core pair, so spreading calls
                    # round-robin parallelizes descriptor generation 4x
                    q0 = bi % 4
                    if Tlo:
                        glo_insts.append(nc.gpsimd.dma_gather(
                            G_t[:, 0:Tlo, :], lo_view,
                            idxlo_t[:, offlo // 16:(offlo + Tlo * 128) // 16],
                            Tlo * 128, Tlo * 128, pay, elem_step=row,
                            single_packet=False, queue_num=q0))
                    if Thi:
                        ghi_insts.append(nc.gpsimd.dma_gather(
                            G_t[:, Tlo:T, :], hi_view,
                            idxhi_t[:, offhi // 16:(offhi + Thi * 128) // 16],
                            Thi * 128, Thi * 128, pay, elem_step=row,
                            single_packet=False, queue_num=(q0 + 1) % 4))
                    if Tlo:
                        ge_insts.append(nc.gpsimd.dma_gather(
                            E_t[:, 0:Tlo, :], ed_view,
                            idxe_t[:, toff * 8:(toff + Tlo) * 8],
                            Tlo * 128, Tlo * 128, 8, elem_step=EROW,
                            single_packet=False, queue_num=(q0 + 2) % 4))
                    if Thi:
                        ge_insts.append(nc.gpsimd.dma_gather(
                            E_t[:, Tlo:T, :], ed_view,
                            idxe_t[:, (toff + Tlo) * 8:(toff + T) * 8],
                            Thi * 128, Thi * 128, 8, elem_step=EROW,
                            single_packet=False, queue_num=(q0 + 3) % 4))

                    consumers = []
                    # S build: S[p,t,j] = (seg[p,t] == j)
                    in0 = seg_t[:, toff:toff + T].to_broadcast([128, T, WINW])
                    jap = J_t[:]
                    in1 = bass.AP(jap.tensor, jap.offset,
                                  [jap.ap[0], [0, T], [1, WINW]])
                    nc.vector.tensor_tensor(out=S_t[:], in0=in0, in1=in1,
                                            op=OP.is_equal)

                    # edge weights x = exp(leaky(e_src + e_dst)); leaky+exp on
                    # the (otherwise idle) scalar engine
                    gf32 = G_t[:].bitcast(dt.float32)   # [128, T, pay//2]
                    ef32 = E_t[:].bitcast(dt.float32)   # [128, T, 4]
                    z_t = gpool.tile([128, T, nh], dt.float32, tag="z")
                    consumers.append(nc.vector.tensor_tensor(
                        out=z_t[:], in0=gf32[:, :, F // 2:F // 2 + nh],
                        in1=ef32[:, :, 0:nh], op=OP.add))
                    nc.vector.scalar_tensor_tensor(
                        out=z_t[:], in0=z_t[:], scalar=NEG_SLOPE, in1=z_t[:],
                        op0=OP.mult, op1=OP.max)
                    # x broadcast-expanded to per-channel lanes so the DVE fold
                    # reads a contiguous operand (2x bf16 rate)
                    x_e = gpool.tile([128, T, nh, CH], dt.bfloat16, tag="xe")
                    zb = bass.AP(z_t[:].tensor, z_t[:].offset,
                                 [z_t[:].ap[0], [nh, T], [1, nh], [0, CH]])
                    nc.scalar.activation(out=x_e[:], in_=zb, func=AF.Exp)
                    consumers.append(nc.scalar.activation(
                        out=G_t[:, :, F:F + nh], in_=z_t[:], func=AF.Exp))

                    # fold x into G (in place)
                    g4 = bass.AP(G_t[:].tensor, G_t[:].offset,
                                 [G_t[:].ap[0], [pay, T], [CH, nh], [1, CH]])
                    consumers.append(nc.vector.tensor_tensor(out=g4, in0=g4,
                                                             in1=x_e[:], op=OP.mult))

                    # matmuls: per chunk psum [128, F+nh]
                    pss = []
                    for cl in range(nb):
                        ep_ps = pspool_e.tile([128, F + nh], dt.float32, tag="ep_ps")
                        pss.append(ep_ps)
                    # tile sequence: lo tiles then hi tiles; stop flag on the
                    # last tile of each (cl, w) across both halves
                    seq = []
                    for tl in (b["lo"], b["hi"]):
                        for (cl, w_, nt) in tl:
                            for k in range(nt):
                                seq.append((cl, w_))
                    last_of = {}
                    for i, kw in enumerate(seq):
                        last_of[kw] = i
                    started = {}
                    for t_id, (cl, w_) in enumerate(seq):
                        keyw = (cl, w_)
                        first = keyw not in started
                        started[keyw] = True
                        consumers.append(nc.tensor.matmul(
                            out=pss[cl][w_ * WINW:(w_ + 1) * WINW, :],
                            lhsT=S_t[:, t_id, :],
                            rhs=G_t[:, t_id, 0:F + nh],
                            start=first, stop=(last_of[keyw] == t_id),
                            tile_position=(0, w_ * WINW),
                            skip_group_check=True))
                    # epilogue per chunk
                    for cl in range(nb):
                        c = b["c0"] + cl
                        ps = pss[cl]
                        inv = epool.tile([128, nh], dt.float32, tag="inv")
                        nc.vector.tensor_scalar_add(out=inv[:], in0=ps[:, F:F + nh],
                                                    scalar1=1e-20)
                        nc.vector.reciprocal(out=inv[:], in_=inv[:])
                        if li < 2:
                            o_sb = epool.tile([128, 256], dt.float32, tag="o_sb")
                        else:
                            o_sb = epool.tile([128, 128], dt.float32, tag="o_sb3")
                            nc.vector.memset(o_sb[:, 64:128], 0.0)
                        for h_ in range(nh):
                            nc.vector.scalar_tensor_tensor(
                                out=o_sb[:, h_ * CH:(h_ + 1) * CH],
                                in0=ps[:, h_ * CH:(h_ + 1) * CH],
                                scalar=inv[:, h_:h_ + 1],
                                in1=b_ts["b" + str(li + 1)][:, h_ * CH:(h_ + 1) * CH],
                                op0=OP.mult, op1=OP.add)
                        nc.scalar.activation(out=o_sb[:, 0:F], in_=o_sb[:, 0:F],
                                             func=AF.Relu)
                        xwd = nc.sync.dma_start(
                            out=L["xout"].ap()[c * 128:(c + 1) * 128, :],
                            in_=o_sb[:])
                        done_q.append((c, xwd))
                    toff += T; offlo += Tlo * 128; offhi += Thi * 128
                for ent in done_q:
                    after_chunk(*ent)
                return glo_insts, ghi_insts, ge_insts

            NLOCH = LOH // 128   # 25 chunks in the lo half
            ag_st = {0: [], 1: [], 2: []}
            ed_st = {0: [], 1: [], 2: []}
            ccs = {}

            def emit_ccs_maybe(li):
                # two half-table AllGathers per layer: lo fires as soon as the
                # first 25 node chunks are written (mid previous edge phase)
                L = LAYERS[li]
                if len(ag_st[li]) == NLOCH and (li, 1) not in ccs:
                    cc1 = nc.gpsimd.collective_compute(
                        "AllGather", OP.bypass, replica_groups=rg,
                        ins=[L["agin"].ap()[0:LOH, :]], outs=[L["tlo"].ap()])
                    for wdm in ag_st[li]:
                        _add_dep_helper(cc1.ins, wdm.ins, sync=True)
                    ccs[(li, 1)] = cc1
                if len(ag_st[li]) == NCHUNK and (li, 2) not in ccs:
                    cc2 = nc.gpsimd.collective_compute(
                        "AllGather", OP.bypass, replica_groups=rg,
                        ins=[L["agin"].ap()[LOH:OWN, :]], outs=[L["thi"].ap()])
                    for wdm in ag_st[li][NLOCH:]:
                        _add_dep_helper(cc2.ins, wdm.ins, sync=True)
                    ccs[(li, 2)] = cc2

            def after_chunk(li, c, xwd):
                # edge phase of layer li just produced chunk c of xs[li+1]
                if li < 2:
                    agd, edd = node_chunk(LAYERS[li + 1], li + 1, c, dep=xwd)
                    ag_st[li + 1].append(agd)
                    ed_st[li + 1].append(edd)
                    emit_ccs_maybe(li + 1)
                else:
                    fc_chunk(c, xwd)

            # layer-1 node phase upfront (reads only the x1T input)
            for c in range(NCHUNK):
                agd, edd = node_chunk(LAYERS[0], 0, c)
                ag_st[0].append(agd)
                ed_st[0].append(edd)
                emit_ccs_maybe(0)

            for li, L in enumerate(LAYERS):
                glo, ghi, ge = edge_phase(
                    L, li, lambda c, xwd, li=li: after_chunk(li, c, xwd))
                for gi in glo:
                    _add_dep_helper(gi.ins, ccs[(li, 1)].ins, sync=True)
                for gi in ghi:
                    _add_dep_helper(gi.ins, ccs[(li, 2)].ins, sync=True)
                for gi in ge:
                    for wdm in ed_st[li]:
                        _add_dep_helper(gi.ins, wdm.ins, sync=True)

    nc.compile()
    return nc


def kernel(x, edge_index, W1, a1_src, a1_dst, b1, W2, a2_src, a2_dst, b2,
           W3, a3_src, a3_dst, b3, fc_W, fc_b):
    x = np.asarray(x, np.float32)
    edge_index = np.asarray(edge_index)
    blocks, per_core = _host_prep(edge_index)

    def ext(Wm, a_s, a_d, nh):
        Wm = np.asarray(Wm, np.float32)
        F = Wm.shape[1]
        A_s = np.zeros((F, nh), np.float32)
        A_d = np.zeros((F, nh), np.float32)
        for h_ in range(nh):
            A_s[h_ * CH:(h_ + 1) * CH, h_] = np.asarray(a_s, np.float32)[h_]
            A_d[h_ * CH:(h_ + 1) * CH, h_] = np.asarray(a_d, np.float32)[h_]
        return np.concatenate([Wm, Wm @ A_s, Wm @ A_d], axis=1)

    W1e = ext(W1, a1_src, a1_dst, H1)                          # [128, 264]
    W2e = ext(W2, a2_src, a2_dst, H2).reshape(2, 128, PAY12)
    W3e = ext(W3, a3_src, a3_dst, H3).reshape(2, 128, PAY3)
    fcWb = np.asarray(fc_W, np.float32)
    J64 = np.broadcast_to(np.arange(WINW, dtype=np.float32), (128, WINW)).astype(bf16)

    def bc(v, w_):
        return np.broadcast_to(np.asarray(v, np.float32)[None, :], (128, w_)).copy()

    xpad = np.zeros((NPAD, IN_DIM), np.float32)
    xpad[:N_NODES] = x

    heads_cfg = dict(pc_shapes={k: per_core[0][k].shape for k in
                                ("idxlo", "idxhi", "idxe", "seg")})
    nc = _build_program(blocks, heads_cfg)

    in_maps = []
    for r in range(W):
        pc = per_core[r]
        in_maps.append({
            "x1T": np.ascontiguousarray(xpad[r * OWN:(r + 1) * OWN].T),
            "idxlo": pc["idxlo"], "idxhi": pc["idxhi"], "idxe": pc["idxe"],
            "seg": pc["seg"],
            "J64": J64, "W1e": W1e, "W2e": W2e, "W3e": W3e, "fcW": fcWb,
            "b1bc": bc(b1, 256), "b2bc": bc(b2, 256), "b3bc": bc(b3, 64),
            "fcbbc": bc(fc_b, N_CLASSES),
        })

    res = bass_utils.run_bass_kernel_spmd(nc, in_maps, core_ids=list(range(W)))
    global LAST_RES
    LAST_RES = res
    out = np.concatenate([res.results[r]["OUT"] for r in range(W)], axis=0)
    return out[:N_NODES].astype(np.float32)


# revision 40
# speedup vs baseline: 1.0603x; 1.0185x over previous
"""3-layer GAT (GATConv x3 + FC) on 8 Trainium2 NeuronCores.

Strategy: dst-sorted edge partitioning (each core owns a contiguous node range
and all edges into it), per-layer node-parallel feature matmul + AllGather of a
gatherable node table [h | e_src], then an edge phase per core: batched
dma_gather of src rows (int16 indices; the table is split into lo/hi halves so
indices fit in int16, and edges are grouped per (chunk, window, half)),
exp(leaky(e_src+e_dst)) edge weights, and segment reduction via one-hot
selection-matrix matmuls accumulated in PSUM per 128-node chunk.  Softmax is
unnormalized (exp without max subtraction) with per-node post-normalization by
the gathered weight sum; self-loops guarantee the sum stays well away from 0.
"""
import os, sys
sys.path.insert(0, '/opt/trn_rl_repo')
import math
import numpy as np
ATH_STAGE = int(os.environ.get("ATH_STAGE", "6"))
ATH_EDGE = int(os.environ.get("ATH_EDGE", "4"))
import ml_dtypes

import concourse.bass as bass
import concourse.bacc as bacc
import concourse.mybir as mybir
import concourse.tile as tile
from concourse import bass_utils
from concourse.bass import _add_dep_helper

# ---- model constants (must match reference.py) ----
NEG_SLOPE = 0.2
H1, H2, H3 = 4, 4, 1
CH = 64
N_NODES = 50000
N_EDGES = 800000
IN_DIM = 128
N_CLASSES = 10

W = 8                    # cores
OWN = 6272               # nodes per core (49 chunks of 128)
NPAD = W * OWN           # 50176
LOH = 3200               # within-core lo-half rows (25 chunks)
NLOROW = W * LOH         # 25600 rows in the lo table (< int16 max)
NHIROW = W * (OWN - LOH) # 24576 rows in the hi table
NCHUNK = OWN // 128      # 49
CPB = 3                  # chunks per gather block
NWIN = 2                 # 64-node windows per chunk
WINW = 64
PAD_SEG = 99.0

# table row layout (bf16 slots)
ROW12 = 384              # layers 1/2: [h(256) | e_src f32 (8 slots) | pad] = 768B stride
PAY12 = 264
ROW3 = 128               # layer 3: [h(64) | e_src f32 (2 slots) | pad] = 256B stride
PAY3 = 66
EROW = 128               # e_dst local table row: 256B stride, leading 8 (or 2) slots = f32

dt = mybir.dt
AF = mybir.ActivationFunctionType
OP = mybir.AluOpType
bf16 = ml_dtypes.bfloat16

# relax dma_gather's elem_size%256 assert (q7 ucode only requires the row
# stride to be a multiple of 256B; elem_size is the per-descriptor length)
import inspect as _inspect, textwrap as _textwrap
_src = _textwrap.dedent(_inspect.getsource(bass.BassGpSimd.dma_gather))
_src = _src.replace("elem_size_bytes > 0 and elem_size_bytes % 256 == 0",
                    "elem_size_bytes > 0")
_ns = dict(bass.__dict__)
exec(compile(_src, "<dma_gather_patched>", "exec"), _ns)
bass.BassGpSimd.dma_gather = _ns["dma_gather"]


def _wrap_idxs(idx_i16):
    """dma_gather index layout: idx i at [i%16, i//16], replicated to 128 parts."""
    n = idx_i16.shape[0]
    cols = n // 16
    arr = idx_i16.reshape(cols, 16).T.copy()
    return np.tile(arr, (8, 1))


def _host_prep(edge_index):
    """Edge sorting/tiling; returns per-core input arrays + static tile metadata.

    Edges (with self-loops) are grouped per (core, chunk, window, srchalf) and
    padded to 128-edge tiles; tile counts are the max over cores so the SPMD
    program is identical everywhere.  Per block (CPB chunks) the tile order is
    all lo-half tiles (groups in order) then all hi-half tiles.
    """
    src = np.concatenate([edge_index[0].astype(np.int64),
                          np.arange(N_NODES, dtype=np.int64)])
    dst = np.concatenate([edge_index[1].astype(np.int64),
                          np.arange(N_NODES, dtype=np.int64)])

    core = dst // OWN
    chunk = (dst % OWN) // 128
    win = ((dst % OWN) % 128) // WINW
    # lo/hi = within-core row halves (so each half AllGathers separately and
    # each table stays under the int16 index limit)
    half = ((src % OWN) >= LOH).astype(np.int64)
    srow = np.where(half == 0,
                    (src // OWN) * LOH + (src % OWN),
                    (src // OWN) * (OWN - LOH) + (src % OWN) - LOH)

    key = (((core * NCHUNK + chunk) * NWIN + win) * 2 + half)
    korder = np.argsort(key, kind="stable")
    src_s, dst_s, srow_s = src[korder], dst[korder], srow[korder]
    ngroups = W * NCHUNK * NWIN * 2
    counts = np.bincount(key[korder], minlength=ngroups).reshape(
        W, NCHUNK, NWIN, 2)
    starts = np.zeros(ngroups + 1, dtype=np.int64)
    np.cumsum(counts.reshape(-1), out=starts[1:])

    tiles_per = np.ceil(counts / 128.0).astype(np.int64).max(axis=0)  # [NCHUNK][NWIN][2]

    blocks = []
    c0 = 0
    while c0 < NCHUNK:
        nb = min(CPB, NCHUNK - c0)
        lo, hi = [], []
        for cl in range(nb):
            for w_ in range(NWIN):
                ntl = int(tiles_per[c0 + cl][w_][0])
                nth = int(tiles_per[c0 + cl][w_][1])
                if ntl:
                    lo.append((cl, w_, ntl))
                if nth:
                    hi.append((cl, w_, nth))
        Tlo = sum(t[2] for t in lo)
        Thi = sum(t[2] for t in hi)
        blocks.append(dict(c0=c0, nb=nb, lo=lo, hi=hi,
                           Tlo=Tlo, Thi=Thi, Tall=Tlo + Thi))
        c0 += nb

    per_core = []
    for r in range(W):
        ilo_cols, ihi_cols, ed_cols, seg_cols = [], [], [], []
        for b in blocks:
            for cls, tl, icols in ((0, b["lo"], ilo_cols), (1, b["hi"], ihi_cols)):
                for (cl, w_, nt) in tl:
                    c = b["c0"] + cl
                    g = (((r * NCHUNK + c) * NWIN + w_) * 2 + cls)
                    s0, s1 = starts[g], starts[g + 1]
                    n_real = s1 - s0
                    cap = nt * 128
                    assert n_real <= cap, (r, c, w_, cls, n_real, cap)
                    pad = cap - n_real
                    icols.append(np.concatenate(
                        [srow_s[s0:s1], np.zeros(pad, np.int64)]))
                    ed_cols.append(np.concatenate(
                        [dst_s[s0:s1] - r * OWN, np.zeros(pad, np.int64)]))
                    seg_cols.append(np.concatenate(
                        [(dst_s[s0:s1] - r * OWN - c * 128 - w_ * WINW).astype(np.float64),
                         np.full(pad, PAD_SEG)]))
        ilo = np.concatenate(ilo_cols).astype(np.int16)
        ihi = np.concatenate(ihi_cols).astype(np.int16)
        ed = np.concatenate(ed_cols).astype(np.int16)
        sg = np.concatenate(seg_cols).reshape(-1, 128).T.astype(bf16)
        per_core.append(dict(idxlo=_wrap_idxs(ilo), idxhi=_wrap_idxs(ihi),
                             idxe=_wrap_idxs(ed),
                             seg=np.ascontiguousarray(sg)))
    return blocks, per_core


def _build_program(blocks, heads_cfg):
    """Build the full 8-core SPMD Bass program. heads_cfg describes layers."""
    nc = bacc.Bacc("TRN2", target_bir_lowering=False, debug=False, num_devices=W,
                   num_swdge_queues=4)
    rg = [list(range(W))]

    NLO = sum(b["Tlo"] for b in blocks) * 128
    NHI = sum(b["Thi"] for b in blocks) * 128
    NT = sum(b["Tall"] for b in blocks)

    # ---------------- inputs ----------------
    pc0_shapes = heads_cfg["pc_shapes"]
    x1T_d = nc.dram_tensor("x1T", [128, OWN], dt.float32, kind="ExternalInput")
    idxlo_d = nc.dram_tensor("idxlo", list(pc0_shapes["idxlo"]), dt.int16, kind="ExternalInput")
    idxhi_d = nc.dram_tensor("idxhi", list(pc0_shapes["idxhi"]), dt.int16, kind="ExternalInput")
    idxe_d = nc.dram_tensor("idxe", list(pc0_shapes["idxe"]), dt.int16, kind="ExternalInput")
    seg_d = nc.dram_tensor("seg", list(pc0_shapes["seg"]), dt.bfloat16, kind="ExternalInput")
    J_d = nc.dram_tensor("J64", [128, WINW], dt.bfloat16, kind="ExternalInput")
    W1e_d = nc.dram_tensor("W1e", [128, PAY12], dt.float32, kind="ExternalInput")
    W2e_d = nc.dram_tensor("W2e", [2, 128, PAY12], dt.float32, kind="ExternalInput")
    W3e_d = nc.dram_tensor("W3e", [2, 128, PAY3], dt.float32, kind="ExternalInput")
    fcW_d = nc.dram_tensor("fcW", [64, N_CLASSES], dt.float32, kind="ExternalInput")
    b1_d = nc.dram_tensor("b1bc", [128, 256], dt.float32, kind="ExternalInput")
    b2_d = nc.dram_tensor("b2bc", [128, 256], dt.float32, kind="ExternalInput")
    b3_d = nc.dram_tensor("b3bc", [128, 64], dt.float32, kind="ExternalInput")
    fcb_d = nc.dram_tensor("fcbbc", [128, N_CLASSES], dt.float32, kind="ExternalInput")
    out_d = nc.dram_tensor("OUT", [OWN, N_CLASSES], dt.float32, kind="ExternalOutput")

    # ---------------- internals ----------------
    tables_lo = [
        nc.dram_tensor("table1lo", [NLOROW, ROW12], dt.bfloat16, kind="Internal", addr_space="Shared"),
        nc.dram_tensor("table2lo", [NLOROW, ROW12], dt.bfloat16, kind="Internal", addr_space="Shared"),
        nc.dram_tensor("table3lo", [NLOROW, ROW3], dt.bfloat16, kind="Internal", addr_space="Shared"),
    ]
    tables_hi = [
        nc.dram_tensor("table1hi", [NHIROW, ROW12], dt.bfloat16, kind="Internal", addr_space="Shared"),
        nc.dram_tensor("table2hi", [NHIROW, ROW12], dt.bfloat16, kind="Internal", addr_space="Shared"),
        nc.dram_tensor("table3hi", [NHIROW, ROW3], dt.bfloat16, kind="Internal", addr_space="Shared"),
    ]
    ag_ins = [
        nc.dram_tensor("agin1", [OWN, ROW12], dt.bfloat16, kind="Internal"),
        nc.dram_tensor("agin2", [OWN, ROW12], dt.bfloat16, kind="Internal"),
        nc.dram_tensor("agin3", [OWN, ROW3], dt.bfloat16, kind="Internal"),
    ]
    edsts = [
        nc.dram_tensor("edst1", [OWN, EROW], dt.bfloat16, kind="Internal"),
        nc.dram_tensor("edst2", [OWN, EROW], dt.bfloat16, kind="Internal"),
        nc.dram_tensor("edst3", [OWN, EROW], dt.bfloat16, kind="Internal"),
    ]
    xs = [
        None,
        nc.dram_tensor("x2", [OWN, 256], dt.float32, kind="Internal"),
        nc.dram_tensor("x3", [OWN, 256], dt.float32, kind="Internal"),
        nc.dram_tensor("x4", [OWN, 128], dt.float32, kind="Internal"),
    ]

    LAYERS = [
        dict(h=H1, F=256, row=ROW12, pay=PAY12, tlo=tables_lo[0], thi=tables_hi[0],
             agin=ag_ins[0], edst=edsts[0], b=b1_d, xout=xs[1], We=W1e_d, nkb=1),
        dict(h=H2, F=256, row=ROW12, pay=PAY12, tlo=tables_lo[1], thi=tables_hi[1],
             agin=ag_ins[1], edst=edsts[1], b=b2_d, xout=xs[2], We=W2e_d, nkb=2),
        dict(h=H3, F=64, row=ROW3, pay=PAY3, tlo=tables_lo[2], thi=tables_hi[2],
             agin=ag_ins[2], edst=edsts[2], b=b3_d, xout=xs[3], We=W3e_d, nkb=2),
    ]

    with tile.TileContext(nc) as tc:
        with tc.tile_pool(name="const", bufs=1) as cpool, \
             tc.tile_pool(name="np_sb", bufs=3) as npool, \
             tc.tile_pool(name="eg", bufs=2) as gpool, \
             tc.tile_pool(name="ep", bufs=3) as epool, \
             tc.tile_pool(name="psum", bufs=2, space="PSUM") as pspool, \
             tc.tile_pool(name="psum_e", bufs=4, space="PSUM") as pspool_e:

            J_t = cpool.tile([128, WINW], dt.bfloat16)
            nc.sync.dma_start(out=J_t[:], in_=J_d.ap())
            # per-edge index / seg streams (identical for all three layers)
            idxlo_t = cpool.tile(list(pc0_shapes["idxlo"]), dt.int16)
            nc.sync.dma_start(out=idxlo_t[:], in_=idxlo_d.ap())
            idxhi_t = cpool.tile(list(pc0_shapes["idxhi"]), dt.int16)
            nc.sync.dma_start(out=idxhi_t[:], in_=idxhi_d.ap())
            idxe_t = cpool.tile(list(pc0_shapes["idxe"]), dt.int16)
            nc.sync.dma_start(out=idxe_t[:], in_=idxe_d.ap())
            seg_t = cpool.tile(list(pc0_shapes["seg"]), dt.bfloat16)
            nc.sync.dma_start(out=seg_t[:], in_=seg_d.ap())

            W1e_t = cpool.tile([128, PAY12], dt.float32)
            nc.sync.dma_start(out=W1e_t[:].bitcast(dt.float32r),
                              in_=W1e_d.ap().bitcast(dt.float32r))
            W2e_t = cpool.tile([128, 2 * PAY12], dt.float32)
            for kb in range(2):
                nc.sync.dma_start(out=W2e_t[:, kb * PAY12:(kb + 1) * PAY12].bitcast(dt.float32r),
                                  in_=W2e_d.ap()[kb].bitcast(dt.float32r))
            W3e_t = cpool.tile([128, 2 * PAY3], dt.float32)
            for kb in range(2):
                nc.sync.dma_start(out=W3e_t[:, kb * PAY3:(kb + 1) * PAY3].bitcast(dt.float32r),
                                  in_=W3e_d.ap()[kb].bitcast(dt.float32r))
            fcW_t = cpool.tile([64, N_CLASSES], dt.float32)
            nc.sync.dma_start(out=fcW_t[:], in_=fcW_d.ap())
            from concourse.masks import make_identity
            ident_t = cpool.tile([128, 128], dt.float32)
            make_identity(nc, ident_t[:])
            b_ts = {}
            for nm, d_, wdt in (("b1", b1_d, 256), ("b2", b2_d, 256),
                                ("b3", b3_d, 64), ("fcb", fcb_d, N_CLASSES)):
                t = cpool.tile([128, wdt], dt.float32, tag=f"bias_{nm}")
                nc.sync.dma_start(out=t[:], in_=d_.ap())
                b_ts[nm] = t

            def node_chunk(L, li, c, dep=None):
                """x @ [W|Wa_src|Wa_dst] for one 128-node chunk -> agin + edst
                rows.  Returns (agin_dma, edst_dma)."""
                F, pay, row = L["F"], L["pay"], L["row"]
                nh = L["h"]
                f32r = dt.float32r
                ps = pspool.tile([128, pay], dt.float32, tag="np_ps")
                if li == 0:
                    lhs = npool.tile([128, 128], dt.float32, tag="np_lhs")
                    nc.sync.dma_start(
                        out=lhs[:].bitcast(f32r),
                        in_=x1T_d.ap()[:, c * 128:(c + 1) * 128].bitcast(f32r))
                    nc.tensor.matmul(out=ps[:], lhsT=lhs[:].bitcast(f32r),
                                     rhs=W1e_t[:].bitcast(f32r),
                                     start=True, stop=True)
                else:
                    xin = xs[li]  # previous layer output [OWN, 256] f32
                    Wt = W2e_t if li == 1 else W3e_t
                    xc = npool.tile([128, 256], dt.float32, tag="np_xc")
                    rd = nc.sync.dma_start(
                        out=xc[:], in_=xin.ap()[c * 128:(c + 1) * 128, :])
                    if dep is not None:
                        _add_dep_helper(rd.ins, dep.ins, sync=True)
                    for kb in range(2):
                        pst = pspool.tile([128, 128], dt.float32, tag="np_tr")
                        nc.tensor.transpose(out=pst[:],
                                            in_=xc[:, kb * 128:(kb + 1) * 128],
                                            identity=ident_t[:])
                        lhs = npool.tile([128, 128], dt.float32, tag="np_lhs")
                        nc.vector.tensor_copy(out=lhs[:].bitcast(f32r), in_=pst[:])
                        nc.tensor.matmul(out=ps[:], lhsT=lhs[:].bitcast(f32r),
                                         rhs=Wt[:, kb * pay:(kb + 1) * pay].bitcast(f32r),
                                         start=(kb == 0), stop=(kb == 1))
                # epilogue: pack row_sb = [h bf16 | e_src f32] ; edst rows
                row_sb = npool.tile([128, row], dt.bfloat16, tag="np_row")
                nc.vector.tensor_copy(out=row_sb[:, 0:F], in_=ps[:, 0:F])
                rf32 = row_sb[:].bitcast(dt.float32)
                nc.vector.tensor_copy(out=rf32[:, F // 2:F // 2 + nh],
                                      in_=ps[:, F:F + nh])
                ed_sb = npool.tile([128, EROW], dt.bfloat16, tag="np_ed")
                ef32 = ed_sb[:].bitcast(dt.float32)
                nc.vector.tensor_copy(out=ef32[:, 0:nh],
                                      in_=ps[:, F + nh:F + 2 * nh])
                agd = nc.sync.dma_start(
                    out=L["agin"].ap()[c * 128:(c + 1) * 128, :], in_=row_sb[:])
                edd = nc.sync.dma_start(
                    out=L["edst"].ap()[c * 128:(c + 1) * 128, :], in_=ed_sb[:])
                return agd, edd

            def fc_chunk(c, dep):
                xc4 = npool.tile([128, 128], dt.float32, tag="fc_xc")
                rd = nc.sync.dma_start(
                    out=xc4[:], in_=xs[3].ap()[c * 128:(c + 1) * 128, 0:128])
                _add_dep_helper(rd.ins, dep.ins, sync=True)
                pst4 = pspool.tile([128, 128], dt.float32, tag="np_tr")
                nc.tensor.transpose(out=pst4[:], in_=xc4[:], identity=ident_t[:])
                lhs = npool.tile([128, 128], dt.float32, tag="fc_lhs")
                nc.scalar.activation(out=lhs[:], in_=pst4[:], func=AF.Copy)
                ps = pspool.tile([128, N_CLASSES], dt.float32, tag="np_ps")
                nc.tensor.matmul(out=ps[:], lhsT=lhs[0:64, :], rhs=fcW_t[:],
                                 start=True, stop=True)
                o_sb = npool.tile([128, N_CLASSES], dt.float32, tag="fc_o")
                nc.vector.tensor_tensor(out=o_sb[:], in0=ps[:],
                                        in1=b_ts["fcb"][:], op=OP.add)
                nc.sync.dma_start(out=out_d.ap()[c * 128:(c + 1) * 128, :],
                                  in_=o_sb[:])

            def edge_phase(L, li, after_chunk):
                """gather + attention + segment-reduce; writes L["xout"].
                after_chunk(c, xw_dma) is invoked as each chunk's output DMA is
                emitted, so the next layer's node work interleaves in program
                order with this layer's edge blocks."""
                F, pay, row, nh = L["F"], L["pay"], L["row"], L["h"]
                eds = L["edst"]
                glo_insts, ghi_insts, ge_insts = [], [], []
                done_q = []   # (chunk, xw_dma) awaiting the lagged after_chunk
                LAG = 2       # blocks of delay so deps are met when emitted
                lo_view = L["tlo"].ap()[:, 0:pay]
                hi_view = L["thi"].ap()[:, 0:pay]
                ed_view = eds.ap()[:, 0:8]
                toff = offlo = offhi = 0
                for bi, b in enumerate(blocks):
                    T, Tlo, Thi = b["Tall"], b["Tlo"], b["Thi"]
                    nb = b["nb"]
                    while len(done_q) > LAG * CPB:
                        after_chunk(*done_q.pop(0))
                    G_t = gpool.tile([128, T, pay], dt.bfloat16, tag="G")
                    E_t = gpool.tile([128, T, 8], dt.bfloat16, tag="E")
                    S_t = gpool.tile([128, T, WINW], dt.bfloat16, tag="S")

                    # four queue-balanced gathers per block: each SWDGE queue
                    # is served by its own Q7 core pair, so spreading calls
                    # round-robin parallelizes descriptor generation 4x
                    q0 = bi % 4
                    # E gathers first: they depend only on local edst writes,
                    # so the in-order Pool sequencer can generate their
                    # descriptors while a table collective is still in flight
                    if Tlo:
                        ge_insts.append(nc.gpsimd.dma_gather(
                            E_t[:, 0:Tlo, :], ed_view,
                            idxe_t[:, toff * 8:(toff + Tlo) * 8],
                            Tlo * 128, Tlo * 128, 8, elem_step=EROW,
                            single_packet=False, queue_num=(q0 + 2) % 4))
                    if Thi:
                        ge_insts.append(nc.gpsimd.dma_gather(
                            E_t[:, Tlo:T, :], ed_view,
                            idxe_t[:, (toff + Tlo) * 8:(toff + T) * 8],
                            Thi * 128, Thi * 128, 8, elem_step=EROW,
                            single_packet=False, queue_num=(q0 + 3) % 4))
                    if Tlo:
                        glo_insts.append(nc.gpsimd.dma_gather(
                            G_t[:, 0:Tlo, :], lo_view,
                            idxlo_t[:, offlo // 16:(offlo + Tlo * 128) // 16],
                            Tlo * 128, Tlo * 128, pay, elem_step=row,
                            single_packet=False, queue_num=q0))
                    if Thi:
                        ghi_insts.append(nc.gpsimd.dma_gather(
                            G_t[:, Tlo:T, :], hi_view,
                            idxhi_t[:, offhi // 16:(offhi + Thi * 128) // 16],
                            Thi * 128, Thi * 128, pay, elem_step=row,
                            single_packet=False, queue_num=(q0 + 1) % 4))

                    consumers = []
                    # S build: S[p,t,j] = (seg[p,t] == j)
                    in0 = seg_t[:, toff:toff + T].to_broadcast([128, T, WINW])
                    jap = J_t[:]
                    in1 = bass.AP(jap.tensor, jap.offset,
                                  [jap.ap[0], [0, T], [1, WINW]])
                    nc.vector.tensor_tensor(out=S_t[:], in0=in0, in1=in1,
                                            op=OP.is_equal)

                    # edge weights x = exp(leaky(e_src + e_dst)); leaky+exp on
                    # the (otherwise idle) scalar engine
                    gf32 = G_t[:].bitcast(dt.float32)   # [128, T, pay//2]
                    ef32 = E_t[:].bitcast(dt.float32)   # [128, T, 4]
                    z_t = gpool.tile([128, T, nh], dt.float32, tag="z")
                    consumers.append(nc.vector.tensor_tensor(
                        out=z_t[:], in0=gf32[:, :, F // 2:F // 2 + nh],
                        in1=ef32[:, :, 0:nh], op=OP.add))
                    nc.vector.scalar_tensor_tensor(
                        out=z_t[:], in0=z_t[:], scalar=NEG_SLOPE, in1=z_t[:],
                        op0=OP.mult, op1=OP.max)
                    # x broadcast-expanded to per-channel lanes so the DVE fold
                    # reads a contiguous operand (2x bf16 rate)
                    x_e = gpool.tile([128, T, nh, CH], dt.bfloat16, tag="xe")
                    zb = bass.AP(z_t[:].tensor, z_t[:].offset,
                                 [z_t[:].ap[0], [nh, T], [1, nh], [0, CH]])
                    nc.scalar.activation(out=x_e[:], in_=zb, func=AF.Exp)
                    consumers.append(nc.scalar.activation(
                        out=G_t[:, :, F:F + nh], in_=z_t[:], func=AF.Exp))

                    # fold x into G (in place)
                    g4 = bass.AP(G_t[:].tensor, G_t[:].offset,
                                 [G_t[:].ap[0], [pay, T], [CH, nh], [1, CH]])
                    consumers.append(nc.vector.tensor_tensor(out=g4, in0=g4,
                                                             in1=x_e[:], op=OP.mult))

                    # matmuls: per chunk psum [128, F+nh]
                    pss = []
                    for cl in range(nb):
                        ep_ps = pspool_e.tile([128, F + nh], dt.float32, tag="ep_ps")
                        pss.append(ep_ps)
                    # tile sequence: lo tiles then hi tiles; stop flag on the
                    # last tile of each (cl, w) across both halves
                    seq = []
                    for tl in (b["lo"], b["hi"]):
                        for (cl, w_, nt) in tl:
                            for k in range(nt):
                                seq.append((cl, w_))
                    last_of = {}
                    for i, kw in enumerate(seq):
                        last_of[kw] = i
                    started = {}
                    for t_id, (cl, w_) in enumerate(seq):
                        keyw = (cl, w_)
                        first = keyw not in started
                        started[keyw] = True
                        consumers.append(nc.tensor.matmul(
                            out=pss[cl][w_ * WINW:(w_ + 1) * WINW, :],
                            lhsT=S_t[:, t_id, :],
                            rhs=G_t[:, t_id, 0:F + nh],
                            start=first, stop=(last_of[keyw] == t_id),
                            tile_position=(0, w_ * WINW),
                            skip_group_check=True))
                    # epilogue per chunk
                    for cl in range(nb):
                        c = b["c0"] + cl
                        ps = pss[cl]
                        inv = epool.tile([128, nh], dt.float32, tag="inv")
                        nc.vector.tensor_scalar_add(out=inv[:], in0=ps[:, F:F + nh],
                                                    scalar1=1e-20)
                        nc.vector.reciprocal(out=inv[:], in_=inv[:])
                        if li < 2:
                            o_sb = epool.tile([128, 256], dt.float32, tag="o_sb")
                        else:
                            o_sb = epool.tile([128, 128], dt.float32, tag="o_sb3")
                            nc.vector.memset(o_sb[:, 64:128], 0.0)
                        for h_ in range(nh):
                            nc.vector.scalar_tensor_tensor(
                                out=o_sb[:, h_ * CH:(h_ + 1) * CH],
                                in0=ps[:, h_ * CH:(h_ + 1) * CH],
                                scalar=inv[:, h_:h_ + 1],
                                in1=b_ts["b" + str(li + 1)][:, h_ * CH:(h_ + 1) * CH],
                                op0=OP.mult, op1=OP.add)
                        nc.scalar.activation(out=o_sb[:, 0:F], in_=o_sb[:, 0:F],
                                             func=AF.Relu)
                        xwd = nc.sync.dma_start(
                            out=L["xout"].ap()[c * 128:(c + 1) * 128, :],
                            in_=o_sb[:])
                        done_q.append((c, xwd))
                    toff += T; offlo += Tlo * 128; offhi += Thi * 128
                for ent in done_q:
                    after_chunk(*ent)
                return glo_insts, ghi_insts, ge_insts

            NLOCH = LOH // 128   # 25 chunks in the lo half
            ag_st = {0: [], 1: [], 2: []}
            ed_st = {0: [], 1: [], 2: []}
            ccs = {}

            def emit_ccs_maybe(li):
                # two half-table AllGathers per layer: lo fires as soon as the
                # first 25 node chunks are written (mid previous edge phase)
                L = LAYERS[li]
                if len(ag_st[li]) == NLOCH and (li, 1) not in ccs:
                    cc1 = nc.gpsimd.collective_compute(
                        "AllGather", OP.bypass, replica_groups=rg,
                        ins=[L["agin"].ap()[0:LOH, :]], outs=[L["tlo"].ap()])
                    for wdm in ag_st[li]:
                        _add_dep_helper(cc1.ins, wdm.ins, sync=True)
                    ccs[(li, 1)] = cc1
                if len(ag_st[li]) == NCHUNK and (li, 2) not in ccs:
                    cc2 = nc.gpsimd.collective_compute(
                        "AllGather", OP.bypass, replica_groups=rg,
                        ins=[L["agin"].ap()[LOH:OWN, :]], outs=[L["thi"].ap()])
                    for wdm in ag_st[li][NLOCH:]:
                        _add_dep_helper(cc2.ins, wdm.ins, sync=True)
                    ccs[(li, 2)] = cc2

            def after_chunk(li, c, xwd):
                # edge phase of layer li just produced chunk c of xs[li+1]
                if li < 2:
                    agd, edd = node_chunk(LAYERS[li + 1], li + 1, c, dep=xwd)
                    ag_st[li + 1].append(agd)
                    ed_st[li + 1].append(edd)
                    emit_ccs_maybe(li + 1)
                else:
                    fc_chunk(c, xwd)

            # layer-1 node phase upfront (reads only the x1T input)
            for c in range(NCHUNK):
                agd, edd = node_chunk(LAYERS[0], 0, c)
                ag_st[0].append(agd)
                ed_st[0].append(edd)
                emit_ccs_maybe(0)

            for li, L in enumerate(LAYERS):
                glo, ghi, ge = edge_phase(
                    L, li, lambda c, xwd, li=li: after_chunk(li, c, xwd))
                for gi in glo:
                    _add_dep_helper(gi.ins, ccs[(li, 1)].ins, sync=True)
                for gi in ghi:
                    _add_dep_helper(gi.ins, ccs[(li, 2)].ins, sync=True)
                for gi in ge:
                    for wdm in ed_st[li]:
                        _add_dep_helper(gi.ins, wdm.ins, sync=True)

    nc.compile()
    return nc


def kernel(x, edge_index, W1, a1_src, a1_dst, b1, W2, a2_src, a2_dst, b2,
           W3, a3_src, a3_dst, b3, fc_W, fc_b):
    x = np.asarray(x, np.float32)
    edge_index = np.asarray(edge_index)
    blocks, per_core = _host_prep(edge_index)

    def ext(Wm, a_s, a_d, nh):
        Wm = np.asarray(Wm, np.float32)
        F = Wm.shape[1]
        A_s = np.zeros((F, nh), np.float32)
        A_d = np.zeros((F, nh), np.float32)
        for h_ in range(nh):
            A_s[h_ * CH:(h_ + 1) * CH, h_] = np.asarray(a_s, np.float32)[h_]
            A_d[h_ * CH:(h_ + 1) * CH, h_] = np.asarray(a_d, np.float32)[h_]
        return np.concatenate([Wm, Wm @ A_s, Wm @ A_d], axis=1)

    W1e = ext(W1, a1_src, a1_dst, H1)                          # [128, 264]
    W2e = ext(W2, a2_src, a2_dst, H2).reshape(2, 128, PAY12)
    W3e = ext(W3, a3_src, a3_dst, H3).reshape(2, 128, PAY3)
    fcWb = np.asarray(fc_W, np.float32)
    J64 = np.broadcast_to(np.arange(WINW, dtype=np.float32), (128, WINW)).astype(bf16)

    def bc(v, w_):
        return np.broadcast_to(np.asarray(v, np.float32)[None, :], (128, w_)).copy()

    xpad = np.zeros((NPAD, IN_DIM), np.float32)
    xpad[:N_NODES] = x

    heads_cfg = dict(pc_shapes={k: per_core[0][k].shape for k in
                                ("idxlo", "idxhi", "idxe", "seg")})
    nc = _build_program(blocks, heads_cfg)

    in_maps = []
    for r in range(W):
        pc = per_core[r]
        in_maps.append({
            "x1T": np.ascontiguousarray(xpad[r * OWN:(r + 1) * OWN].T),
            "idxlo": pc["idxlo"], "idxhi": pc["idxhi"], "idxe": pc["idxe"],
            "seg": pc["seg"],
            "J64": J64, "W1e": W1e, "W2e": W2e, "W3e": W3e, "fcW": fcWb,
            "b1bc": bc(b1, 256), "b2bc": bc(b2, 256), "b3bc": bc(b3, 64),
            "fcbbc": bc(fc_b, N_CLASSES),
        })

    res = bass_utils.run_bass_kernel_spmd(nc, in_maps, core_ids=list(range(W)))
    global LAST_RES
    LAST_RES = res
    out = np.concatenate([res.results[r]["OUT"] for r in range(W)], axis=0)
    return out[:N_NODES].astype(np.float32)
